# revision 11
# baseline (speedup 1.0000x reference)
"""Trainium2 Bass kernel for DynamicResidualStageWrapper (18-block MLP stage
with channel-gated anchor routing), data-parallel over batch across 8 cores.

Contract: kernel(**inputs) takes FULL unsharded inputs (as numpy arrays, keyed
as in reference.setup_inputs()) and returns the FULL output [32,14,14,512].

Per-core layout: activations live transposed as [C=512, tokens=784] split into
4 partition-tiles [128, 784]; tokens are (sample b, position hw) with 2 token
chunks of 392 (= 2 samples) per PSUM bank. Block weights [cin, cout] are the
natural lhsT for out[cout, tok] = W.T @ X, so there are no transposes anywhere.
Block matmuls run in float32r (full-rate fp32 mode, inputs pre-rounded to the
bf16 hi+lo representable set); the tiny router MLP runs in plain fp32.
Softmax's exp uses e^x = (1+tanh(x/2))/(1-tanh(x/2)) so the scalar engine
never has to switch activation-table sets away from gelu/tanh.
"""

import os
import numpy as np

import concourse.bacc as bacc
import concourse.bass as bass
import concourse.mybir as mybir
import concourse.tile as tile
from concourse.bass_utils import run_bass_kernel_spmd

# ---- problem constants (hardcoded per spec) ----
NUM_BLOCKS = 18
ANCHOR_IDX = (1, 4, 9)
TARGET_IDX = (11, 14, 17)
C = 512
HID = 128
A = 3
B, H, W = 32, 14, 14
N_CORES = 8
BL = B // N_CORES          # 4 samples per core
HW = H * W                 # 196 positions per sample
T = BL * HW                # 784 tokens per core
KT = C // 128              # 4 channel tiles
NCH = 2                    # token chunks (PSUM banks) per row
CH = T // NCH              # 392 tokens per chunk
PSW = 1024                 # psum tile width: chunk c at [c*512, c*512+CH)

F32 = mybir.dt.float32
USE_F32R = os.environ.get("KBENCH_MM_DT", "f32r") == "f32r"
MM = mybir.dt.float32r if USE_F32R else F32
GELU = mybir.ActivationFunctionType.Gelu_apprx_tanh
TANH = mybir.ActivationFunctionType.Tanh
BF16 = mybir.dt.bfloat16
# which channel-tiles' routing update runs on gpsimd (rest on vector)
GPS_KS = tuple(int(x) for x in os.environ.get("KBENCH_GPS_KS", "").split(",")
               if x != "")

_cached = {}


def build_program():
    """Build the per-core Bass/Tile program (same program on all 8 cores)."""
    nc = bacc.Bacc(trn_type="TRN2", target_bir_lowering=False, debug=False)

    xT = nc.dram_tensor("xT", [C, T], MM, kind="ExternalInput").ap()
    wd = nc.dram_tensor("wd", [NUM_BLOCKS, C, C], MM, kind="ExternalInput").ap()
    bias_cols = nc.dram_tensor("bias_cols", [128, NUM_BLOCKS * KT], F32,
                               kind="ExternalInput").ap()
    fc1w = nc.dram_tensor("fc1w", [128, A * KT * 128], BF16, kind="ExternalInput").ap()
    fc1b = nc.dram_tensor("fc1b", [128, A], F32, kind="ExternalInput").ap()
    fc2w = nc.dram_tensor("fc2w", [128, A * A * C], BF16, kind="ExternalInput").ap()
    fc2bias = nc.dram_tensor("fc2bias", [128, A * A * KT * BL], F32,
                             kind="ExternalInput").ap()
    gbc = nc.dram_tensor("gbc", [128, A], F32, kind="ExternalInput").ap()
    outT = nc.dram_tensor("outT", [C, T], MM, kind="ExternalOutput").ap()

    anchor_of = {b: i for i, b in enumerate(ANCHOR_IDX)}
    target_of = {b: i for i, b in enumerate(TARGET_IDX)}

    with tile.TileContext(nc) as tc:
        with (
            tc.tile_pool(name="const", bufs=1) as cpool,
            tc.tile_pool(name="wpool", bufs=6) as wpool,
            tc.tile_pool(name="xpool", bufs=3) as xpool,
            tc.tile_pool(name="apool", bufs=1) as apool,
            tc.tile_pool(name="rpool", bufs=2) as rpool,
            tc.tile_pool(name="ppool", bufs=3, space="PSUM") as ppool,
            tc.tile_pool(name="fcps", bufs=1, space="PSUM") as fcps,
        ):
            # ---- startup DMAs spread across queues: inputs + first-block
            # bias on the ACT hwdge queue (idle at startup), block weights on
            # the SP queue, router constants on the gpsimd swdge queue.
            X = []
            w_t0 = wpool.tile([128, KT * C], MM, tag="w", name="w0")
            for k in range(KT):
                nc.sync.dma_start(w_t0[:, k * C:(k + 1) * C],
                                  wd[0, k * 128:(k + 1) * 128, :])
                xt = xpool.tile([128, T], MM, tag=f"x{k}", name=f"xin{k}")
                nc.sync.dma_start(xt[:], xT[k * 128:(k + 1) * 128, :])
                X.append(xt)
            bias_t = cpool.tile([128, NUM_BLOCKS * KT], F32, name="bias_t")
            nc.scalar.dma_start(bias_t[:], bias_cols[:])
            fc1b_t = cpool.tile([128, A], F32, name="fc1b_t")
            nc.gpsimd.dma_start(fc1b_t[:], fc1b[:])
            gbc_t = cpool.tile([128, A], F32, name="gbc_t")
            nc.gpsimd.dma_start(gbc_t[:], gbc[:])
            # per-target fc weights are DMA'd mid-run (3 blocks ahead of use)
            fc1w_t, fc2w_t, fc2bias_t = {}, {}, {}

            anchors = {}   # a -> [tile per k]

            for i in range(NUM_BLOCKS):
                t_idx = target_of.get(i)
                a_idx = anchor_of.get(i)

                # prefetch the router weights for a target ~3 blocks out
                if i + 3 in target_of:
                    tt = target_of[i + 3]
                    f1 = cpool.tile([128, KT * 128], BF16, name=f"fc1w_{tt}")
                    nc.gpsimd.dma_start(
                        f1[:], fc1w[:, tt * KT * 128:(tt + 1) * KT * 128])
                    fc1w_t[tt] = f1
                    f2 = cpool.tile([128, A * C], BF16, name=f"fc2w_{tt}")
                    nc.gpsimd.dma_start(
                        f2[:], fc2w[:, tt * A * C:(tt + 1) * A * C])
                    fc2w_t[tt] = f2
                    fb = cpool.tile([128, A * KT * BL], F32, name=f"fc2b_{tt}")
                    nc.gpsimd.dma_start(
                        fb[:], fc2bias[:, tt * A * KT * BL:(tt + 1) * A * KT * BL])
                    fc2bias_t[tt] = fb

                # block weights: lhsT slice for (k, ct) at cols k*512 + ct*128
                if i == 0:
                    w_t = w_t0
                else:
                    w_t = wpool.tile([128, KT * C], MM, tag="w", name=f"w{i}")
                    for k in range(KT):
                        nc.sync.dma_start(w_t[:, k * C:(k + 1) * C],
                                          wd[i, k * 128:(k + 1) * 128, :])

                Xn = []
                for ct in range(KT):
                    ps = ppool.tile([128, PSW], F32, tag="mm", name=f"ps{i}_{ct}")
                    for c in range(NCH):
                        for k in range(KT):
                            nc.tensor.matmul(
                                ps[:, c * 512:c * 512 + CH],
                                w_t[:, k * C + ct * 128:k * C + (ct + 1) * 128],
                                X[k][:, c * CH:(c + 1) * CH],
                                start=(k == 0), stop=(k == KT - 1))
                    if a_idx is not None:
                        xn = apool.tile([128, T], MM, tag=f"a{a_idx}_{ct}",
                                        name=f"anc{a_idx}_{ct}")
                    else:
                        xn = xpool.tile([128, T], MM, tag=f"x{ct}",
                                        name=f"xb{i}_{ct}")
                    ps_v = ps[:].rearrange("p (c m) -> p c m", c=NCH)[:, :, 0:CH]
                    xn_v = xn[:].rearrange("p (c m) -> p c m", c=NCH)
                    nc.scalar.activation(
                        xn_v, ps_v, GELU,
                        bias=bias_t[:, i * KT + ct:i * KT + ct + 1])
                    Xn.append(xn)

                if a_idx is not None:
                    anchors[a_idx] = Xn
                    if a_idx == 2:
                        # precompute anchor differences (gates sum to gamma:
                        # routed = gamma*a2 + g0*(a0-a2) + g1*(a1-a2))
                        adiff = {}
                        for da in range(2):
                            adiff[da] = []
                            for k in range(KT):
                                dt_ = apool.tile([128, T], F32,
                                                 tag=f"ad{da}_{k}",
                                                 name=f"ad{da}_{k}")
                                nc.vector.tensor_sub(dt_[:], anchors[da][k][:],
                                                     anchors[2][k][:])
                                adiff[da].append(dt_)
                if t_idx is not None:
                    Xn = _routing(nc, rpool, xpool, fcps, t_idx, Xn, anchors,
                                  adiff, fc1w_t, fc1b_t, fc2w_t, fc2bias_t,
                                  gbc_t, outT if i == NUM_BLOCKS - 1 else None)
                X = Xn

    nc.compile()
    return nc


def _routing(nc, rpool, xpool, fcps, t, Xn, anchors, adiff,
             fc1w_t, fc1b_t, fc2w_t, fc2bias_t, gbc_t, outT=None):
    """ChannelGating router: mean-pool -> 2-layer MLP -> softmax over anchors
    -> weighted anchor sum added to Xn. Returns the updated activation tiles."""
    mul = mybir.AluOpType.mult
    add = mybir.AluOpType.add

    # mean pool (the 1/196 divisor is folded into fc1w host-side), then
    # round to f32r for the fc1 matmul
    pooled = []
    for k in range(KT):
        pl = rpool.tile([128, BL], F32, tag=f"pool{k}", name=f"pool{t}_{k}")
        nc.vector.reduce_sum(pl[:], Xn[k][:].rearrange("p (b m) -> p b m", b=BL),
                             axis=mybir.AxisListType.X)
        pr = rpool.tile([128, BL], BF16, tag=f"poolr{k}", name=f"poolr{t}_{k}")
        nc.vector.tensor_copy(pr[:], pl[:])
        pooled.append(pr)

    # fc1: h = gelu(pooled @ fc1_w + fc1_b)   [HID=128, BL]
    ps1 = fcps.tile([128, BL], F32, tag="fcps", name=f"ps1_{t}")
    for k in range(KT):
        nc.tensor.matmul(ps1[:], fc1w_t[t][:, k * 128:(k + 1) * 128], pooled[k][:],
                         start=(k == 0), stop=(k == KT - 1))
    h = rpool.tile([128, BL], BF16, tag="h", name=f"h_{t}")
    nc.scalar.activation(h[:], ps1[:], GELU, bias=fc1b_t[:, t:t + 1])

    # fc2: logits [A*C, BL] as 12 col-tiles of one [128, 48] psum
    NJ = A * KT  # 12
    ps2 = fcps.tile([128, NJ * BL], F32, tag="fcps", name=f"ps2_{t}")
    for j in range(NJ):
        nc.tensor.matmul(ps2[:, j * BL:(j + 1) * BL],
                         fc2w_t[t][:, j * 128:(j + 1) * 128],
                         h[:], start=True, stop=True)
    logits = rpool.tile([128, NJ * BL], F32, tag="logits", name=f"lg_{t}")
    nc.vector.tensor_add(logits[:], ps2[:], fc2bias_t[t][:])

    # softmax over a (cols = a*16 + k*4 + b), exp via tanh identity:
    # e^x = (1 + tanh(x/2)) / (1 - tanh(x/2)); logits are O(0.1) here so
    # the max-subtraction is skipped (tanh path is stable to |x|~17)
    KB = KT * BL  # 16
    th = rpool.tile([128, A * KB], F32, tag="th", name=f"th_{t}")
    nc.scalar.activation(th[:], logits[:], TANH, scale=0.5)
    num = rpool.tile([128, A * KB], F32, tag="num", name=f"num_{t}")
    nc.vector.tensor_scalar_add(num[:], th[:], 1.0)
    den = rpool.tile([128, A * KB], F32, tag="den", name=f"den_{t}")
    nc.vector.tensor_scalar(den[:], th[:], -1.0, 1.0, op0=mul, op1=add)
    rec = rpool.tile([128, A * KB], F32, tag="rec", name=f"rec_{t}")
    nc.vector.reciprocal(rec[:], den[:])
    e = rpool.tile([128, A * KB], F32, tag="e", name=f"e_{t}")
    nc.vector.tensor_mul(e[:], num[:], rec[:])
    s = rpool.tile([128, KB], F32, tag="s", name=f"s_{t}")
    nc.vector.tensor_reduce(s[:], e[:].rearrange("p (a kb) -> p kb a", a=A),
                            axis=mybir.AxisListType.X, op=add)
    rinv = rpool.tile([128, KB], F32, tag="rinv", name=f"rinv_{t}")
    nc.vector.reciprocal(rinv[:], s[:])
    rg = rpool.tile([128, KB], F32, tag="rg", name=f"rg_{t}")
    nc.vector.tensor_scalar_mul(rg[:], rinv[:], gbc_t[:, t:t + 1])
    g = rpool.tile([128, 2 * KB], F32, tag="g", name=f"g_{t}")
    for a in range(2):
        nc.vector.tensor_mul(g[:, a * KB:(a + 1) * KB],
                             e[:, a * KB:(a + 1) * KB], rg[:])

    # weighted anchor sum: xr = Xn + sum_a g_a * anchor_a, per sample;
    # fused multiply-add via scalar_tensor_tensor, split across DVE + gpsimd
    Xr = []
    for k in range(KT):
        xr = xpool.tile([128, T], MM, tag=f"x{k}", name=f"xr{t}_{k}")
        # base term: xr = Xn + gamma * a2   (one full-width op per k)
        nc.vector.scalar_tensor_tensor(
            xr[:], anchors[2][k][:], gbc_t[:, t:t + 1], Xn[k][:],
            op0=mul, op1=add)
        # per-sample correction: xr += g0*(a0-a2) + g1*(a1-a2)
        for b in range(BL):
            sl = slice(b * HW, (b + 1) * HW)
            for a in range(2):
                col = a * KB + k * BL + b
                nc.vector.scalar_tensor_tensor(
                    xr[:, sl], adiff[a][k][:, sl],
                    g[:, col:col + 1], xr[:, sl], op0=mul, op1=add)
            if outT is not None:
                nc.sync.dma_start(outT[k * 128:(k + 1) * 128, sl], xr[:, sl])
        Xr.append(xr)
    return Xr


def _round_f32r(a):
    """Round fp32 to the f32r-representable set (bf16 hi + bf16 lo)."""
    if not USE_F32R:
        return np.ascontiguousarray(a, dtype=np.float32)
    import ml_dtypes
    a = np.asarray(a, dtype=np.float32)
    hi = a.astype(ml_dtypes.bfloat16).astype(np.float32)
    lo = (a - hi).astype(ml_dtypes.bfloat16).astype(np.float32)
    return np.ascontiguousarray(hi + lo)


def _prep_shared(block_w, block_b, fc1_w, fc1_b, fc2_w, fc2_b, gammas):
    """Host-side packing of the (replicated) weight tensors."""
    f = np.float32
    wd = np.ascontiguousarray(block_w, dtype=f)
    # bias column (i*KT+ct) = block_b[i, ct*128:(ct+1)*128]
    bias_cols = np.ascontiguousarray(
        block_b.reshape(NUM_BLOCKS * KT, 128).T, dtype=f)
    # fc1 with the mean-pool divisor folded in; col block (t*KT+k)
    fc1s = (fc1_w / float(HW)).astype(f)                      # [A, C, HID]
    fc1w_cat = np.concatenate(
        [fc1s[t][k * 128:(k + 1) * 128, :] for t in range(A) for k in range(KT)],
        axis=1)                                               # [128, A*KT*128]
    fc1b_cols = np.ascontiguousarray(np.asarray(fc1_b, dtype=f).T)  # [128, A]
    fc2w_cat = np.concatenate([np.asarray(fc2_w[t], dtype=f) for t in range(A)],
                              axis=1)                          # [128, A*A*C]
    # fc2 bias expanded to the [128, (a,k,b)] logits layout, repeated per b
    fc2bias = np.concatenate(
        [np.repeat(np.asarray(fc2_b[t], dtype=f).reshape(A * KT, 128).T,
                   BL, axis=1) for t in range(A)], axis=1)     # [128, A*A*KT*BL]
    gbc = np.broadcast_to(np.asarray(gammas, dtype=f)[None, :], (128, A))
    gbc = np.ascontiguousarray(gbc)
    import ml_dtypes
    return dict(wd=_round_f32r(wd), bias_cols=np.ascontiguousarray(bias_cols),
                fc1w=np.ascontiguousarray(fc1w_cat.astype(ml_dtypes.bfloat16)),
                fc1b=fc1b_cols,
                fc2w=np.ascontiguousarray(fc2w_cat.astype(ml_dtypes.bfloat16)),
                fc2bias=np.ascontiguousarray(fc2bias), gbc=gbc)


def shard_x(x):
    """Full x [B,H,W,C] -> per-core transposed shards [C, T]."""
    shards = []
    for r in range(N_CORES):
        xs = np.asarray(x[r * BL:(r + 1) * BL], dtype=np.float32)  # [BL,H,W,C]
        shards.append(_round_f32r(xs.reshape(T, C).T))             # [C, T]
    return shards


def unshard_out(outs):
    """Per-core [C, T] results -> full [B,H,W,C]."""
    parts = [o.T.reshape(BL, H, W, C) for o in outs]
    return np.ascontiguousarray(np.concatenate(parts, axis=0), dtype=np.float32)


def kernel(x, block_w, block_b, fc1_w, fc1_b, fc2_w, fc2_b, gammas):
    if "nc" not in _cached:
        _cached["nc"] = build_program()
    nc = _cached["nc"]

    shared = _prep_shared(block_w, block_b, fc1_w, fc1_b, fc2_w, fc2_b, gammas)
    xs = shard_x(x)
    in_maps = [dict(shared, xT=xs[r]) for r in range(N_CORES)]
    res = run_bass_kernel_spmd(nc, in_maps, list(range(N_CORES)))
    return unshard_out([res.results[r]["outT"] for r in range(N_CORES)])


# revision 13
# speedup vs baseline: 1.0964x; 1.0964x over previous
"""Trainium2 Bass kernel for DynamicResidualStageWrapper (18-block MLP stage
with channel-gated anchor routing), data-parallel over batch across 8 cores.

Contract: kernel(**inputs) takes FULL unsharded inputs (as numpy arrays, keyed
as in reference.setup_inputs()) and returns the FULL output [32,14,14,512].

Per-core layout: activations live transposed as [C=512, tokens=784] split into
4 partition-tiles [128, 784]; tokens are (sample b, position hw) with 2 token
chunks of 392 (= 2 samples) per PSUM bank. Block weights [cin, cout] are the
natural lhsT for out[cout, tok] = W.T @ X, so there are no transposes anywhere.
Block matmuls run in float32r (full-rate fp32 mode, inputs pre-rounded to the
bf16 hi+lo representable set); the tiny router MLP runs in plain fp32.
Softmax's exp uses e^x = (1+tanh(x/2))/(1-tanh(x/2)) so the scalar engine
never has to switch activation-table sets away from gelu/tanh.
"""

import os
import numpy as np

import concourse.bacc as bacc
import concourse.bass as bass
import concourse.mybir as mybir
import concourse.tile as tile
from concourse.bass_utils import run_bass_kernel_spmd

# ---- problem constants (hardcoded per spec) ----
NUM_BLOCKS = 18
ANCHOR_IDX = (1, 4, 9)
TARGET_IDX = (11, 14, 17)
C = 512
HID = 128
A = 3
B, H, W = 32, 14, 14
N_CORES = 8
BL = B // N_CORES          # 4 samples per core
HW = H * W                 # 196 positions per sample
T = BL * HW                # 784 tokens per core
KT = C // 128              # 4 channel tiles
NCH = 2                    # token chunks (PSUM banks) per row
CH = T // NCH              # 392 tokens per chunk
PSW = 1024                 # psum tile width: chunk c at [c*512, c*512+CH)

F32 = mybir.dt.float32
USE_F32R = os.environ.get("KBENCH_MM_DT", "f32r") == "f32r"
MM = mybir.dt.float32r if USE_F32R else F32
GELU = mybir.ActivationFunctionType.Gelu_apprx_tanh
TANH = mybir.ActivationFunctionType.Tanh
BF16 = mybir.dt.bfloat16
# which channel-tiles' routing update runs on gpsimd (rest on vector)
GPS_KS = tuple(int(x) for x in os.environ.get("KBENCH_GPS_KS", "").split(",")
               if x != "")

_cached = {}


def build_program():
    """Build the per-core Bass/Tile program (same program on all 8 cores)."""
    nc = bacc.Bacc(trn_type="TRN2", target_bir_lowering=False, debug=False)

    xT = nc.dram_tensor("xT", [C, T], MM, kind="ExternalInput").ap()
    wd = nc.dram_tensor("wd", [NUM_BLOCKS, C, C], MM, kind="ExternalInput").ap()
    bias_cols = nc.dram_tensor("bias_cols", [128, NUM_BLOCKS * KT], F32,
                               kind="ExternalInput").ap()
    fc1w = nc.dram_tensor("fc1w", [128, A * KT * 128], BF16, kind="ExternalInput").ap()
    fc1b = nc.dram_tensor("fc1b", [128, A], F32, kind="ExternalInput").ap()
    fc2w = nc.dram_tensor("fc2w", [128, A * A * C], BF16, kind="ExternalInput").ap()
    fc2bias = nc.dram_tensor("fc2bias", [128, A * A * KT * BL], F32,
                             kind="ExternalInput").ap()
    gbc = nc.dram_tensor("gbc", [128, A], F32, kind="ExternalInput").ap()
    outT = nc.dram_tensor("outT", [C, T], MM, kind="ExternalOutput").ap()

    anchor_of = {b: i for i, b in enumerate(ANCHOR_IDX)}
    target_of = {b: i for i, b in enumerate(TARGET_IDX)}

    with tile.TileContext(nc) as tc:
        with (
            tc.tile_pool(name="const", bufs=1) as cpool,
            tc.tile_pool(name="wpool", bufs=6) as wpool,
            tc.tile_pool(name="xpool", bufs=3) as xpool,
            tc.tile_pool(name="apool", bufs=1) as apool,
            tc.tile_pool(name="rpool", bufs=2) as rpool,
            tc.tile_pool(name="ppool", bufs=6, space="PSUM") as ppool,
            tc.tile_pool(name="fcps", bufs=1, space="PSUM") as fcps,
        ):
            # ---- startup DMAs spread across queues: inputs + first-block
            # bias on the ACT hwdge queue (idle at startup), block weights on
            # the SP queue, router constants on the gpsimd swdge queue.
            X = []
            for k in range(KT):
                xt = xpool.tile([128, T], MM, tag=f"x{k}", name=f"xin{k}")
                eng = nc.scalar if k < 2 else nc.gpsimd
                eng.dma_start(xt[:], xT[k * 128:(k + 1) * 128, :])
                X.append(xt)
            bias_t = cpool.tile([128, NUM_BLOCKS * KT], F32, name="bias_t")
            nc.scalar.dma_start(bias_t[:], bias_cols[:])
            fc1b_t = cpool.tile([128, A], F32, name="fc1b_t")
            nc.gpsimd.dma_start(fc1b_t[:], fc1b[:])
            gbc_t = cpool.tile([128, A], F32, name="gbc_t")
            nc.gpsimd.dma_start(gbc_t[:], gbc[:])
            # per-target fc weights are DMA'd mid-run (3 blocks ahead of use)
            fc1w_t, fc2w_t, fc2bias_t = {}, {}, {}

            anchors = {}   # a -> [tile per k]

            for i in range(NUM_BLOCKS):
                t_idx = target_of.get(i)
                a_idx = anchor_of.get(i)

                # prefetch the router weights for a target ~3 blocks out
                if i + 3 in target_of:
                    tt = target_of[i + 3]
                    f1 = cpool.tile([128, KT * 128], BF16, name=f"fc1w_{tt}")
                    nc.gpsimd.dma_start(
                        f1[:], fc1w[:, tt * KT * 128:(tt + 1) * KT * 128])
                    fc1w_t[tt] = f1
                    f2 = cpool.tile([128, A * C], BF16, name=f"fc2w_{tt}")
                    nc.gpsimd.dma_start(
                        f2[:], fc2w[:, tt * A * C:(tt + 1) * A * C])
                    fc2w_t[tt] = f2
                    fb = cpool.tile([128, A * KT * BL], F32, name=f"fc2b_{tt}")
                    nc.gpsimd.dma_start(
                        fb[:], fc2bias[:, tt * A * KT * BL:(tt + 1) * A * KT * BL])
                    fc2bias_t[tt] = fb

                # block weights: lhsT slice for (k, ct) at cols k*512 + ct*128
                w_t = wpool.tile([128, KT * C], MM, tag="w", name=f"w{i}")
                for k in range(KT):
                    nc.sync.dma_start(w_t[:, k * C:(k + 1) * C],
                                      wd[i, k * 128:(k + 1) * 128, :])

                Xn = []
                for ct in range(KT):
                    if a_idx is not None:
                        xn = apool.tile([128, T], MM, tag=f"a{a_idx}_{ct}",
                                        name=f"anc{a_idx}_{ct}")
                    else:
                        xn = xpool.tile([128, T], MM, tag=f"x{ct}",
                                        name=f"xb{i}_{ct}")
                    for c in range(NCH):
                        ps = ppool.tile([128, 512], F32, tag="mm",
                                        name=f"ps{i}_{ct}_{c}")
                        for k in range(KT):
                            nc.tensor.matmul(
                                ps[:, 0:CH],
                                w_t[:, k * C + ct * 128:k * C + (ct + 1) * 128],
                                X[k][:, c * CH:(c + 1) * CH],
                                start=(k == 0), stop=(k == KT - 1))
                        nc.scalar.activation(
                            xn[:, c * CH:(c + 1) * CH], ps[:, 0:CH], GELU,
                            bias=bias_t[:, i * KT + ct:i * KT + ct + 1])
                    Xn.append(xn)

                if a_idx is not None:
                    anchors[a_idx] = Xn
                    if a_idx == 2:
                        # precompute anchor differences (gates sum to gamma:
                        # routed = gamma*a2 + g0*(a0-a2) + g1*(a1-a2))
                        adiff = {}
                        for da in range(2):
                            adiff[da] = []
                            for k in range(KT):
                                dt_ = apool.tile([128, T], F32,
                                                 tag=f"ad{da}_{k}",
                                                 name=f"ad{da}_{k}")
                                nc.vector.tensor_sub(dt_[:], anchors[da][k][:],
                                                     anchors[2][k][:])
                                adiff[da].append(dt_)
                if t_idx is not None:
                    Xn = _routing(nc, rpool, xpool, fcps, t_idx, Xn, anchors,
                                  adiff, fc1w_t, fc1b_t, fc2w_t, fc2bias_t,
                                  gbc_t, outT if i == NUM_BLOCKS - 1 else None)
                X = Xn

    nc.compile()
    return nc


def _routing(nc, rpool, xpool, fcps, t, Xn, anchors, adiff,
             fc1w_t, fc1b_t, fc2w_t, fc2bias_t, gbc_t, outT=None):
    """ChannelGating router: mean-pool -> 2-layer MLP -> softmax over anchors
    -> weighted anchor sum added to Xn. Returns the updated activation tiles."""
    mul = mybir.AluOpType.mult
    add = mybir.AluOpType.add

    # mean pool (the 1/196 divisor is folded into fc1w host-side), then
    # round to f32r for the fc1 matmul
    pooled = []
    for k in range(KT):
        pl = rpool.tile([128, BL], F32, tag=f"pool{k}", name=f"pool{t}_{k}")
        nc.vector.reduce_sum(pl[:], Xn[k][:].rearrange("p (b m) -> p b m", b=BL),
                             axis=mybir.AxisListType.X)
        pr = rpool.tile([128, BL], BF16, tag=f"poolr{k}", name=f"poolr{t}_{k}")
        nc.vector.tensor_copy(pr[:], pl[:])
        pooled.append(pr)

    # fc1: h = gelu(pooled @ fc1_w + fc1_b)   [HID=128, BL]
    ps1 = fcps.tile([128, BL], F32, tag="fcps", name=f"ps1_{t}")
    for k in range(KT):
        nc.tensor.matmul(ps1[:], fc1w_t[t][:, k * 128:(k + 1) * 128], pooled[k][:],
                         start=(k == 0), stop=(k == KT - 1))
    h = rpool.tile([128, BL], BF16, tag="h", name=f"h_{t}")
    nc.scalar.activation(h[:], ps1[:], GELU, bias=fc1b_t[:, t:t + 1])

    # fc2: logits [A*C, BL] as 12 col-tiles of one [128, 48] psum
    NJ = A * KT  # 12
    ps2 = fcps.tile([128, NJ * BL], F32, tag="fcps", name=f"ps2_{t}")
    for j in range(NJ):
        nc.tensor.matmul(ps2[:, j * BL:(j + 1) * BL],
                         fc2w_t[t][:, j * 128:(j + 1) * 128],
                         h[:], start=True, stop=True)
    logits = rpool.tile([128, NJ * BL], F32, tag="logits", name=f"lg_{t}")
    nc.vector.tensor_add(logits[:], ps2[:], fc2bias_t[t][:])

    # softmax over a (cols = a*16 + k*4 + b), exp via tanh identity:
    # e^x = (1 + tanh(x/2)) / (1 - tanh(x/2)); logits are O(0.1) here so
    # the max-subtraction is skipped (tanh path is stable to |x|~17)
    KB = KT * BL  # 16
    th = rpool.tile([128, A * KB], F32, tag="th", name=f"th_{t}")
    nc.scalar.activation(th[:], logits[:], TANH, scale=0.5)
    num = rpool.tile([128, A * KB], F32, tag="num", name=f"num_{t}")
    nc.vector.tensor_scalar_add(num[:], th[:], 1.0)
    den = rpool.tile([128, A * KB], F32, tag="den", name=f"den_{t}")
    nc.vector.tensor_scalar(den[:], th[:], -1.0, 1.0, op0=mul, op1=add)
    rec = rpool.tile([128, A * KB], F32, tag="rec", name=f"rec_{t}")
    nc.vector.reciprocal(rec[:], den[:])
    e = rpool.tile([128, A * KB], F32, tag="e", name=f"e_{t}")
    nc.vector.tensor_mul(e[:], num[:], rec[:])
    s = rpool.tile([128, KB], F32, tag="s", name=f"s_{t}")
    nc.vector.tensor_reduce(s[:], e[:].rearrange("p (a kb) -> p kb a", a=A),
                            axis=mybir.AxisListType.X, op=add)
    rinv = rpool.tile([128, KB], F32, tag="rinv", name=f"rinv_{t}")
    nc.vector.reciprocal(rinv[:], s[:])
    rg = rpool.tile([128, KB], F32, tag="rg", name=f"rg_{t}")
    nc.vector.tensor_scalar_mul(rg[:], rinv[:], gbc_t[:, t:t + 1])
    g = rpool.tile([128, 2 * KB], F32, tag="g", name=f"g_{t}")
    for a in range(2):
        nc.vector.tensor_mul(g[:, a * KB:(a + 1) * KB],
                             e[:, a * KB:(a + 1) * KB], rg[:])

    # weighted anchor sum: xr = Xn + sum_a g_a * anchor_a, per sample;
    # fused multiply-add via scalar_tensor_tensor, split across DVE + gpsimd
    Xr = []
    for k in range(KT):
        xr = xpool.tile([128, T], MM, tag=f"x{k}", name=f"xr{t}_{k}")
        # base term: xr = Xn + gamma * a2   (one full-width op per k)
        nc.vector.scalar_tensor_tensor(
            xr[:], anchors[2][k][:], gbc_t[:, t:t + 1], Xn[k][:],
            op0=mul, op1=add)
        # per-sample correction: xr += g0*(a0-a2) + g1*(a1-a2)
        for b in range(BL):
            sl = slice(b * HW, (b + 1) * HW)
            for a in range(2):
                col = a * KB + k * BL + b
                nc.vector.scalar_tensor_tensor(
                    xr[:, sl], adiff[a][k][:, sl],
                    g[:, col:col + 1], xr[:, sl], op0=mul, op1=add)
        if outT is not None:
            nc.sync.dma_start(outT[k * 128:(k + 1) * 128, :], xr[:])
        Xr.append(xr)
    return Xr


def _round_f32r(a):
    """Round fp32 to the f32r-representable set (bf16 hi + bf16 lo)."""
    if not USE_F32R:
        return np.ascontiguousarray(a, dtype=np.float32)
    import ml_dtypes
    a = np.asarray(a, dtype=np.float32)
    hi = a.astype(ml_dtypes.bfloat16).astype(np.float32)
    lo = (a - hi).astype(ml_dtypes.bfloat16).astype(np.float32)
    return np.ascontiguousarray(hi + lo)


def _prep_shared(block_w, block_b, fc1_w, fc1_b, fc2_w, fc2_b, gammas):
    """Host-side packing of the (replicated) weight tensors."""
    f = np.float32
    wd = np.ascontiguousarray(block_w, dtype=f)
    # bias column (i*KT+ct) = block_b[i, ct*128:(ct+1)*128]
    bias_cols = np.ascontiguousarray(
        block_b.reshape(NUM_BLOCKS * KT, 128).T, dtype=f)
    # fc1 with the mean-pool divisor folded in; col block (t*KT+k)
    fc1s = (fc1_w / float(HW)).astype(f)                      # [A, C, HID]
    fc1w_cat = np.concatenate(
        [fc1s[t][k * 128:(k + 1) * 128, :] for t in range(A) for k in range(KT)],
        axis=1)                                               # [128, A*KT*128]
    fc1b_cols = np.ascontiguousarray(np.asarray(fc1_b, dtype=f).T)  # [128, A]
    fc2w_cat = np.concatenate([np.asarray(fc2_w[t], dtype=f) for t in range(A)],
                              axis=1)                          # [128, A*A*C]
    # fc2 bias expanded to the [128, (a,k,b)] logits layout, repeated per b
    fc2bias = np.concatenate(
        [np.repeat(np.asarray(fc2_b[t], dtype=f).reshape(A * KT, 128).T,
                   BL, axis=1) for t in range(A)], axis=1)     # [128, A*A*KT*BL]
    gbc = np.broadcast_to(np.asarray(gammas, dtype=f)[None, :], (128, A))
    gbc = np.ascontiguousarray(gbc)
    import ml_dtypes
    return dict(wd=_round_f32r(wd), bias_cols=np.ascontiguousarray(bias_cols),
                fc1w=np.ascontiguousarray(fc1w_cat.astype(ml_dtypes.bfloat16)),
                fc1b=fc1b_cols,
                fc2w=np.ascontiguousarray(fc2w_cat.astype(ml_dtypes.bfloat16)),
                fc2bias=np.ascontiguousarray(fc2bias), gbc=gbc)


def shard_x(x):
    """Full x [B,H,W,C] -> per-core transposed shards [C, T]."""
    shards = []
    for r in range(N_CORES):
        xs = np.asarray(x[r * BL:(r + 1) * BL], dtype=np.float32)  # [BL,H,W,C]
        shards.append(_round_f32r(xs.reshape(T, C).T))             # [C, T]
    return shards


def unshard_out(outs):
    """Per-core [C, T] results -> full [B,H,W,C]."""
    parts = [o.T.reshape(BL, H, W, C) for o in outs]
    return np.ascontiguousarray(np.concatenate(parts, axis=0), dtype=np.float32)


def kernel(x, block_w, block_b, fc1_w, fc1_b, fc2_w, fc2_b, gammas):
    if "nc" not in _cached:
        _cached["nc"] = build_program()
    nc = _cached["nc"]

    shared = _prep_shared(block_w, block_b, fc1_w, fc1_b, fc2_w, fc2_b, gammas)
    xs = shard_x(x)
    in_maps = [dict(shared, xT=xs[r]) for r in range(N_CORES)]
    res = run_bass_kernel_spmd(nc, in_maps, list(range(N_CORES)))
    return unshard_out([res.results[r]["outT"] for r in range(N_CORES)])


# revision 14
# speedup vs baseline: 1.1072x; 1.0099x over previous
"""Trainium2 Bass kernel for DynamicResidualStageWrapper (18-block MLP stage
with channel-gated anchor routing), data-parallel over batch across 8 cores.

Contract: kernel(**inputs) takes FULL unsharded inputs (as numpy arrays, keyed
as in reference.setup_inputs()) and returns the FULL output [32,14,14,512].

Per-core layout: activations live transposed as [C=512, tokens=784] split into
4 partition-tiles [128, 784]; tokens are (sample b, position hw) with 2 token
chunks of 392 (= 2 samples) per PSUM bank. Block weights [cin, cout] are the
natural lhsT for out[cout, tok] = W.T @ X, so there are no transposes anywhere.
Block matmuls run in float32r (full-rate fp32 mode, inputs pre-rounded to the
bf16 hi+lo representable set); the tiny router MLP runs in plain fp32.
Softmax's exp uses e^x = (1+tanh(x/2))/(1-tanh(x/2)) so the scalar engine
never has to switch activation-table sets away from gelu/tanh.
"""

import os
import numpy as np

import concourse.bacc as bacc
import concourse.bass as bass
import concourse.mybir as mybir
import concourse.tile as tile
from concourse.bass_utils import run_bass_kernel_spmd

# ---- problem constants (hardcoded per spec) ----
NUM_BLOCKS = 18
ANCHOR_IDX = (1, 4, 9)
TARGET_IDX = (11, 14, 17)
C = 512
HID = 128
A = 3
B, H, W = 32, 14, 14
N_CORES = 8
BL = B // N_CORES          # 4 samples per core
HW = H * W                 # 196 positions per sample
T = BL * HW                # 784 tokens per core
KT = C // 128              # 4 channel tiles
NCH = 2                    # token chunks (PSUM banks) per row
CH = T // NCH              # 392 tokens per chunk
PSW = 1024                 # psum tile width: chunk c at [c*512, c*512+CH)

F32 = mybir.dt.float32
USE_F32R = os.environ.get("KBENCH_MM_DT", "f32r") == "f32r"
MM = mybir.dt.float32r if USE_F32R else F32
GELU = mybir.ActivationFunctionType.Gelu_apprx_tanh
TANH = mybir.ActivationFunctionType.Tanh
BF16 = mybir.dt.bfloat16
# which channel-tiles' routing update runs on gpsimd (rest on vector)
GPS_KS = tuple(int(x) for x in os.environ.get("KBENCH_GPS_KS", "").split(",")
               if x != "")

_cached = {}


def build_program():
    """Build the per-core Bass/Tile program (same program on all 8 cores)."""
    nc = bacc.Bacc(trn_type="TRN2", target_bir_lowering=False, debug=False)

    xT = nc.dram_tensor("xT", [C, T], MM, kind="ExternalInput").ap()
    wd = nc.dram_tensor("wd", [NUM_BLOCKS, C, C], MM, kind="ExternalInput").ap()
    bias_cols = nc.dram_tensor("bias_cols", [128, NUM_BLOCKS * KT], F32,
                               kind="ExternalInput").ap()
    fc1w = nc.dram_tensor("fc1w", [128, A * KT * 128], BF16, kind="ExternalInput").ap()
    fc1b = nc.dram_tensor("fc1b", [128, A], F32, kind="ExternalInput").ap()
    fc2w = nc.dram_tensor("fc2w", [128, A * A * C], BF16, kind="ExternalInput").ap()
    fc2bias = nc.dram_tensor("fc2bias", [128, A * A * KT * BL], F32,
                             kind="ExternalInput").ap()
    gbc = nc.dram_tensor("gbc", [128, A], F32, kind="ExternalInput").ap()
    outT = nc.dram_tensor("outT", [C, T], MM, kind="ExternalOutput").ap()

    anchor_of = {b: i for i, b in enumerate(ANCHOR_IDX)}
    target_of = {b: i for i, b in enumerate(TARGET_IDX)}

    with tile.TileContext(nc) as tc:
        with (
            tc.tile_pool(name="const", bufs=1) as cpool,
            tc.tile_pool(name="wpool", bufs=8) as wpool,
            tc.tile_pool(name="xpool", bufs=3) as xpool,
            tc.tile_pool(name="apool", bufs=1) as apool,
            tc.tile_pool(name="rpool", bufs=2) as rpool,
            tc.tile_pool(name="ppool", bufs=7, space="PSUM") as ppool,
            tc.tile_pool(name="fcps", bufs=1, space="PSUM") as fcps,
        ):
            # ---- startup DMAs spread across queues: inputs + first-block
            # bias on the ACT hwdge queue (idle at startup), block weights on
            # the SP queue, router constants on the gpsimd swdge queue.
            X = []
            for k in range(KT):
                xt = xpool.tile([128, T], MM, tag=f"x{k}", name=f"xin{k}")
                eng = nc.scalar if k < 2 else nc.gpsimd
                eng.dma_start(xt[:], xT[k * 128:(k + 1) * 128, :])
                X.append(xt)
            bias_t = cpool.tile([128, NUM_BLOCKS * KT], F32, name="bias_t")
            nc.scalar.dma_start(bias_t[:], bias_cols[:])
            fc1b_t = cpool.tile([128, A], F32, name="fc1b_t")
            nc.gpsimd.dma_start(fc1b_t[:], fc1b[:])
            gbc_t = cpool.tile([128, A], F32, name="gbc_t")
            nc.gpsimd.dma_start(gbc_t[:], gbc[:])
            # per-target fc weights are DMA'd mid-run (3 blocks ahead of use)
            fc1w_t, fc2w_t, fc2bias_t = {}, {}, {}

            anchors = {}   # a -> [tile per k]

            for i in range(NUM_BLOCKS):
                t_idx = target_of.get(i)
                a_idx = anchor_of.get(i)

                # prefetch the router weights for a target ~3 blocks out
                if i + 3 in target_of:
                    tt = target_of[i + 3]
                    f1 = cpool.tile([128, KT * 128], BF16, name=f"fc1w_{tt}")
                    nc.gpsimd.dma_start(
                        f1[:], fc1w[:, tt * KT * 128:(tt + 1) * KT * 128])
                    fc1w_t[tt] = f1
                    f2 = cpool.tile([128, A * C], BF16, name=f"fc2w_{tt}")
                    nc.gpsimd.dma_start(
                        f2[:], fc2w[:, tt * A * C:(tt + 1) * A * C])
                    fc2w_t[tt] = f2
                    fb = cpool.tile([128, A * KT * BL], F32, name=f"fc2b_{tt}")
                    nc.gpsimd.dma_start(
                        fb[:], fc2bias[:, tt * A * KT * BL:(tt + 1) * A * KT * BL])
                    fc2bias_t[tt] = fb

                # block weights: lhsT slice for (k, ct) at cols k*512 + ct*128
                w_t = wpool.tile([128, KT * C], MM, tag="w", name=f"w{i}")
                for k in range(KT):
                    nc.sync.dma_start(w_t[:, k * C:(k + 1) * C],
                                      wd[i, k * 128:(k + 1) * 128, :])

                Xn = []
                for ct in range(KT):
                    if a_idx is not None:
                        xn = apool.tile([128, T], MM, tag=f"a{a_idx}_{ct}",
                                        name=f"anc{a_idx}_{ct}")
                    else:
                        xn = xpool.tile([128, T], MM, tag=f"x{ct}",
                                        name=f"xb{i}_{ct}")
                    for c in range(NCH):
                        ps = ppool.tile([128, 512], F32, tag="mm",
                                        name=f"ps{i}_{ct}_{c}")
                        for k in range(KT):
                            nc.tensor.matmul(
                                ps[:, 0:CH],
                                w_t[:, k * C + ct * 128:k * C + (ct + 1) * 128],
                                X[k][:, c * CH:(c + 1) * CH],
                                start=(k == 0), stop=(k == KT - 1))
                        nc.scalar.activation(
                            xn[:, c * CH:(c + 1) * CH], ps[:, 0:CH], GELU,
                            bias=bias_t[:, i * KT + ct:i * KT + ct + 1])
                    Xn.append(xn)

                if a_idx is not None:
                    anchors[a_idx] = Xn
                    if a_idx == 2:
                        # precompute anchor differences (gates sum to gamma:
                        # routed = gamma*a2 + g0*(a0-a2) + g1*(a1-a2))
                        adiff = {}
                        for da in range(2):
                            adiff[da] = []
                            for k in range(KT):
                                dt_ = apool.tile([128, T], F32,
                                                 tag=f"ad{da}_{k}",
                                                 name=f"ad{da}_{k}")
                                nc.vector.tensor_sub(dt_[:], anchors[da][k][:],
                                                     anchors[2][k][:])
                                adiff[da].append(dt_)
                if t_idx is not None:
                    Xn = _routing(nc, rpool, xpool, fcps, t_idx, Xn, anchors,
                                  adiff, fc1w_t, fc1b_t, fc2w_t, fc2bias_t,
                                  gbc_t, outT if i == NUM_BLOCKS - 1 else None)
                X = Xn

    nc.compile()
    return nc


def _routing(nc, rpool, xpool, fcps, t, Xn, anchors, adiff,
             fc1w_t, fc1b_t, fc2w_t, fc2bias_t, gbc_t, outT=None):
    """ChannelGating router: mean-pool -> 2-layer MLP -> softmax over anchors
    -> weighted anchor sum added to Xn. Returns the updated activation tiles."""
    mul = mybir.AluOpType.mult
    add = mybir.AluOpType.add

    # mean pool (the 1/196 divisor is folded into fc1w host-side), then
    # round to f32r for the fc1 matmul
    pooled = []
    for k in range(KT):
        pl = rpool.tile([128, BL], F32, tag=f"pool{k}", name=f"pool{t}_{k}")
        nc.vector.reduce_sum(pl[:], Xn[k][:].rearrange("p (b m) -> p b m", b=BL),
                             axis=mybir.AxisListType.X)
        pr = rpool.tile([128, BL], BF16, tag=f"poolr{k}", name=f"poolr{t}_{k}")
        nc.vector.tensor_copy(pr[:], pl[:])
        pooled.append(pr)

    # fc1: h = gelu(pooled @ fc1_w + fc1_b)   [HID=128, BL]
    ps1 = fcps.tile([128, BL], F32, tag="fcps", name=f"ps1_{t}")
    for k in range(KT):
        nc.tensor.matmul(ps1[:], fc1w_t[t][:, k * 128:(k + 1) * 128], pooled[k][:],
                         start=(k == 0), stop=(k == KT - 1))
    h = rpool.tile([128, BL], BF16, tag="h", name=f"h_{t}")
    nc.scalar.activation(h[:], ps1[:], GELU, bias=fc1b_t[:, t:t + 1])

    # fc2: logits [A*C, BL] as 12 col-tiles of one [128, 48] psum
    NJ = A * KT  # 12
    ps2 = fcps.tile([128, NJ * BL], F32, tag="fcps", name=f"ps2_{t}")
    for j in range(NJ):
        nc.tensor.matmul(ps2[:, j * BL:(j + 1) * BL],
                         fc2w_t[t][:, j * 128:(j + 1) * 128],
                         h[:], start=True, stop=True)
    logits = rpool.tile([128, NJ * BL], F32, tag="logits", name=f"lg_{t}")
    nc.vector.tensor_add(logits[:], ps2[:], fc2bias_t[t][:])

    # softmax over a (cols = a*16 + k*4 + b), exp via tanh identity:
    # e^x = (1 + tanh(x/2)) / (1 - tanh(x/2)); logits are O(0.1) here so
    # the max-subtraction is skipped (tanh path is stable to |x|~17)
    KB = KT * BL  # 16
    th = rpool.tile([128, A * KB], F32, tag="th", name=f"th_{t}")
    nc.scalar.activation(th[:], logits[:], TANH, scale=0.5)
    num = rpool.tile([128, A * KB], F32, tag="num", name=f"num_{t}")
    nc.vector.tensor_scalar_add(num[:], th[:], 1.0)
    den = rpool.tile([128, A * KB], F32, tag="den", name=f"den_{t}")
    nc.vector.tensor_scalar(den[:], th[:], -1.0, 1.0, op0=mul, op1=add)
    rec = rpool.tile([128, A * KB], F32, tag="rec", name=f"rec_{t}")
    nc.vector.reciprocal(rec[:], den[:])
    e = rpool.tile([128, A * KB], F32, tag="e", name=f"e_{t}")
    nc.vector.tensor_mul(e[:], num[:], rec[:])
    s = rpool.tile([128, KB], F32, tag="s", name=f"s_{t}")
    nc.vector.tensor_reduce(s[:], e[:].rearrange("p (a kb) -> p kb a", a=A),
                            axis=mybir.AxisListType.X, op=add)
    rinv = rpool.tile([128, KB], F32, tag="rinv", name=f"rinv_{t}")
    nc.vector.reciprocal(rinv[:], s[:])
    rg = rpool.tile([128, KB], F32, tag="rg", name=f"rg_{t}")
    nc.vector.tensor_scalar_mul(rg[:], rinv[:], gbc_t[:, t:t + 1])
    g = rpool.tile([128, 2 * KB], F32, tag="g", name=f"g_{t}")
    for a in range(2):
        nc.vector.tensor_mul(g[:, a * KB:(a + 1) * KB],
                             e[:, a * KB:(a + 1) * KB], rg[:])

    # weighted anchor sum: xr = Xn + sum_a g_a * anchor_a, per sample;
    # fused multiply-add via scalar_tensor_tensor, split across DVE + gpsimd
    Xr = []
    for k in range(KT):
        xr = xpool.tile([128, T], MM, tag=f"x{k}", name=f"xr{t}_{k}")
        # base term: xr = Xn + gamma * a2   (one full-width op per k)
        nc.vector.scalar_tensor_tensor(
            xr[:], anchors[2][k][:], gbc_t[:, t:t + 1], Xn[k][:],
            op0=mul, op1=add)
        # per-sample correction: xr += g0*(a0-a2) + g1*(a1-a2)
        for b in range(BL):
            sl = slice(b * HW, (b + 1) * HW)
            for a in range(2):
                col = a * KB + k * BL + b
                nc.vector.scalar_tensor_tensor(
                    xr[:, sl], adiff[a][k][:, sl],
                    g[:, col:col + 1], xr[:, sl], op0=mul, op1=add)
        if outT is not None:
            nc.sync.dma_start(outT[k * 128:(k + 1) * 128, :], xr[:])
        Xr.append(xr)
    return Xr


def _round_f32r(a):
    """Round fp32 to the f32r-representable set (bf16 hi + bf16 lo)."""
    if not USE_F32R:
        return np.ascontiguousarray(a, dtype=np.float32)
    import ml_dtypes
    a = np.asarray(a, dtype=np.float32)
    hi = a.astype(ml_dtypes.bfloat16).astype(np.float32)
    lo = (a - hi).astype(ml_dtypes.bfloat16).astype(np.float32)
    return np.ascontiguousarray(hi + lo)


def _prep_shared(block_w, block_b, fc1_w, fc1_b, fc2_w, fc2_b, gammas):
    """Host-side packing of the (replicated) weight tensors."""
    f = np.float32
    wd = np.ascontiguousarray(block_w, dtype=f)
    # bias column (i*KT+ct) = block_b[i, ct*128:(ct+1)*128]
    bias_cols = np.ascontiguousarray(
        block_b.reshape(NUM_BLOCKS * KT, 128).T, dtype=f)
    # fc1 with the mean-pool divisor folded in; col block (t*KT+k)
    fc1s = (fc1_w / float(HW)).astype(f)                      # [A, C, HID]
    fc1w_cat = np.concatenate(
        [fc1s[t][k * 128:(k + 1) * 128, :] for t in range(A) for k in range(KT)],
        axis=1)                                               # [128, A*KT*128]
    fc1b_cols = np.ascontiguousarray(np.asarray(fc1_b, dtype=f).T)  # [128, A]
    fc2w_cat = np.concatenate([np.asarray(fc2_w[t], dtype=f) for t in range(A)],
                              axis=1)                          # [128, A*A*C]
    # fc2 bias expanded to the [128, (a,k,b)] logits layout, repeated per b
    fc2bias = np.concatenate(
        [np.repeat(np.asarray(fc2_b[t], dtype=f).reshape(A * KT, 128).T,
                   BL, axis=1) for t in range(A)], axis=1)     # [128, A*A*KT*BL]
    gbc = np.broadcast_to(np.asarray(gammas, dtype=f)[None, :], (128, A))
    gbc = np.ascontiguousarray(gbc)
    import ml_dtypes
    return dict(wd=_round_f32r(wd), bias_cols=np.ascontiguousarray(bias_cols),
                fc1w=np.ascontiguousarray(fc1w_cat.astype(ml_dtypes.bfloat16)),
                fc1b=fc1b_cols,
                fc2w=np.ascontiguousarray(fc2w_cat.astype(ml_dtypes.bfloat16)),
                fc2bias=np.ascontiguousarray(fc2bias), gbc=gbc)


def shard_x(x):
    """Full x [B,H,W,C] -> per-core transposed shards [C, T]."""
    shards = []
    for r in range(N_CORES):
        xs = np.asarray(x[r * BL:(r + 1) * BL], dtype=np.float32)  # [BL,H,W,C]
        shards.append(_round_f32r(xs.reshape(T, C).T))             # [C, T]
    return shards


def unshard_out(outs):
    """Per-core [C, T] results -> full [B,H,W,C]."""
    parts = [o.T.reshape(BL, H, W, C) for o in outs]
    return np.ascontiguousarray(np.concatenate(parts, axis=0), dtype=np.float32)


def kernel(x, block_w, block_b, fc1_w, fc1_b, fc2_w, fc2_b, gammas):
    if "nc" not in _cached:
        _cached["nc"] = build_program()
    nc = _cached["nc"]

    shared = _prep_shared(block_w, block_b, fc1_w, fc1_b, fc2_w, fc2_b, gammas)
    xs = shard_x(x)
    in_maps = [dict(shared, xT=xs[r]) for r in range(N_CORES)]
    res = run_bass_kernel_spmd(nc, in_maps, list(range(N_CORES)))
    return unshard_out([res.results[r]["outT"] for r in range(N_CORES)])
